# revision 42
# baseline (speedup 1.0000x reference)
"""Trainium2 Bass kernel: RMSNorm + RoPE + causal attention + output projection.

Tensor-parallel over heads: 16 heads / 8 cores = 2 heads per core.
Each core computes a full [S, D] partial output (its heads' contribution to
the 'snh,dnh->sd' projection); the all-reduce is done host-side in the gather.

Layout strategy (per core):
  - h^T [D, S] built by PE-transposing RMSNorm'd xs tiles.
  - Q^T/K^T/V^T [HD, S] per head via weight-stationary fp32r matmuls.
  - RoPE applied on Q^T/K^T with host-permuted head dims (even dims ->
    partitions 0-63, odd -> 64-127) so pair mixing is two half-width DVE ops
    (one operand in PSUM to satisfy the equal-base-partition rule).
  - Scores computed transposed: S^T[t, s] per (kv-tile, q-chunk); softmax
    denominators via ones[128,128]-stationary matmul (accumulates broadcast
    column sums in PSUM); causal handled by restricting column ranges plus a
    single triangular mask on diagonal tiles.
  - PV: lhsT = V natural tiles -> O^T accumulated in PSUM, normalized by
    reciprocal of the broadcast denominators.
  - Output projection: attn^T tiles stationary, wo^T moving.
"""
import os
import sys
import types

import numpy as np

SEQ, D, NH, HD = 4096, 2048, 16, 128
NCORES = 8
HPC = NH // NCORES          # heads per core
M = HPC * HD                # per-core fused head dim (256)
EPS = 1e-6
ROPE_BASE = 10000.0
SM_SCALE = 1.0 / np.sqrt(HD)
CHUNK = 512                 # q-chunk (free dim of score matmuls)
NCHUNK = SEQ // CHUNK       # 8
NT = SEQ // 128             # 32 s-tiles
DT = D // 128               # 16 d-tiles


def _inject_ntff_hook():
    """Register the axon NTFF profiling hook (missing antenv.axon_hooks)."""
    if "antenv.axon_hooks" in sys.modules:
        return
    try:
        import antenv
        from trn_agent_boot.trn_boot import _ntff_profile_via_ctypes
    except ImportError:
        return
    holder = [None]
    mod = types.ModuleType("antenv.axon_hooks")
    mod.set_axon_ntff_profile_hook = lambda h: holder.__setitem__(0, h)
    mod.get_axon_ntff_profile_hook = lambda: holder[0]
    sys.modules["antenv.axon_hooks"] = mod
    antenv.axon_hooks = mod
    try:
        mod.set_axon_ntff_profile_hook(
            _ntff_profile_via_ctypes("/opt/axon/libaxon_pjrt.so"))
    except Exception:
        pass


def _build_nc():
    import concourse.bass as bass  # noqa: F401
    import concourse.mybir as mybir
    import concourse.tile as tile
    from concourse import bacc

    FP32 = mybir.dt.float32
    FP32R = mybir.dt.float32r
    AF = mybir.ActivationFunctionType
    ALU = mybir.AluOpType

    nc = bacc.Bacc(None, target_bir_lowering=False)

    xs = nc.declare_dram_parameter("xs", [SEQ, D], FP32, isOutput=False)
    wq = nc.declare_dram_parameter("wq", [D, M], FP32R, isOutput=False)
    wk = nc.declare_dram_parameter("wk", [D, M], FP32R, isOutput=False)
    wv = nc.declare_dram_parameter("wv", [D, M], FP32R, isOutput=False)
    wo = nc.declare_dram_parameter("wo", [D, M], FP32R, isOutput=False)
    cosd = nc.declare_dram_parameter("cosd", [128, SEQ], FP32, isOutput=False)
    sind = nc.declare_dram_parameter("sind", [128, SEQ], FP32, isOutput=False)
    tri = nc.declare_dram_parameter("tri", [128, 128], FP32R, isOutput=False)
    ones = nc.declare_dram_parameter("ones", [128, 128], FP32R, isOutput=False)
    ident = nc.declare_dram_parameter("ident", [128, 128], FP32, isOutput=False)
    identr = nc.declare_dram_parameter("identr", [128, 128], FP32R, isOutput=False)
    out = nc.declare_dram_parameter("out", [SEQ, D], FP32, isOutput=True)

    # DRAM scratch: per-head Q^T/K^T (rope'd) and V^T, all fp32r
    qt_d = nc.dram_tensor("qt_d", [HPC, 128, SEQ], FP32R)
    kt_d = nc.dram_tensor("kt_d", [HPC, 128, SEQ], FP32R)
    vt_d = nc.dram_tensor("vt_d", [HPC, 128, SEQ], FP32R)
    
    with tile.TileContext(nc) as tc:
        with tc.tile_pool(name="consts", bufs=1) as consts:
            tri_sb = consts.tile([128, 128], FP32R)
            nc.sync.dma_start(out=tri_sb[:], in_=tri[:])
            ones_sb = consts.tile([128, 128], FP32R)
            nc.sync.dma_start(out=ones_sb[:], in_=ones[:])
            id_sb = consts.tile([128, 128], FP32)
            nc.sync.dma_start(out=id_sb[:], in_=ident[:])
            idr_sb = consts.tile([128, 128], FP32R)
            nc.sync.dma_start(out=idr_sb[:], in_=identr[:])
            eps_sb = consts.tile([128, 1], FP32)
            nc.vector.memset(eps_sb[:], EPS)

            # ---------------- Phase 1: norm + h^T + QKV projections + RoPE
            with tc.tile_pool(name="p1w", bufs=1) as p1w, \
                 tc.tile_pool(name="p1ht", bufs=1) as p1ht, \
                 tc.tile_pool(name="p1x", bufs=3) as p1x, \
                 tc.tile_pool(name="p1s", bufs=1) as p1s, \
                 tc.tile_pool(name="p1n", bufs=8) as p1n, \
                 tc.tile_pool(name="p1st", bufs=3) as p1st, \
                 tc.tile_pool(name="p1ps_t", bufs=2, space="PSUM") as p1ps_t, \
                 tc.tile_pool(name="p1ps_p", bufs=2, space="PSUM") as p1ps_p, \
                 tc.tile_pool(name="p1ps_r", bufs=2, space="PSUM") as p1ps_r:
                xt_pre = []
                for st4 in range(4):
                    xt = p1x.tile([128, D], FP32, name="xt")
                    nc.sync.dma_start(out=xt[:], in_=xs[st4 * 128:(st4 + 1) * 128, :])
                    xt_pre.append(xt)
                wq_sb = p1w.tile([128, DT * M], FP32R)
                nc.sync.dma_start(out=wq_sb[:].rearrange("p (t m) -> p t m", t=DT),
                                  in_=wq[:].rearrange("(t p) m -> p t m", p=128))
                for st4 in range(4):
                    xt = p1x.tile([128, D], FP32, name="xt")
                    nc.sync.dma_start(out=xt[:],
                                      in_=xs[(4 + st4) * 128:(5 + st4) * 128, :])
                    xt_pre.append(xt)
                wk_sb = p1w.tile([128, DT * M], FP32R)
                nc.sync.dma_start(out=wk_sb[:].rearrange("p (t m) -> p t m", t=DT),
                                  in_=wk[:].rearrange("(t p) m -> p t m", p=128))
                wv_sb = p1w.tile([128, DT * M], FP32R)
                nc.sync.dma_start(out=wv_sb[:].rearrange("p (t m) -> p t m", t=DT),
                                  in_=wv[:].rearrange("(t p) m -> p t m", p=128))
                cos_sb = p1w.tile([128, SEQ], FP32)
                nc.sync.dma_start(out=cos_sb[:], in_=cosd[:])
                sin_sb = p1w.tile([128, SEQ], FP32)
                nc.sync.dma_start(out=sin_sb[:], in_=sind[:])

                for c in range(NCHUNK):
                    # -- load 4 s-tiles, RMSNorm in place
                    h_tiles = []
                    for st4 in range(4):
                        st = c * 4 + st4
                        if c < 2:
                            xt = xt_pre[c * 4 + st4]
                        else:
                            xt = p1x.tile([128, D], FP32, name="xt")
                            nc.sync.dma_start(out=xt[:], in_=xs[st * 128:(st + 1) * 128, :])
                        sq = p1s.tile([128, D], FP32, name="sq")
                        ssq = p1n.tile([128, 1], FP32, name="ssq")
                        nc.scalar.activation(sq[:], xt[:], AF.Square,
                                             accum_out=ssq[:])
                        std = p1n.tile([128, 1], FP32, name="std")
                        nc.scalar.activation(std[:], ssq[:], AF.Sqrt,
                                             scale=1.0 / D, bias=eps_sb[:])
                        istd = p1n.tile([128, 1], FP32, name="istd")
                        nc.vector.reciprocal(istd[:], std[:])
                        hr = p1x.tile([128, D], FP32R, name="hr", bufs=4)
                        nc.vector.tensor_scalar_mul(hr[:], xt[:], istd[:])
                        h_tiles.append(hr)

                    # -- transpose into h^T chunk [128, DT*CHUNK] fp32r
                    ht = p1ht.tile([128, DT * CHUNK], FP32R, name="ht")
                    for dt2 in range(DT // 2):
                        tp = p1ps_t.tile([128, 2 * CHUNK], FP32R, name="tp")
                        for half in range(2):
                            dt = dt2 * 2 + half
                            for st4 in range(4):
                                nc.tensor.transpose(
                                    tp[:, half * CHUNK + st4 * 128:
                                       half * CHUNK + (st4 + 1) * 128],
                                    h_tiles[st4][:, dt * 128:(dt + 1) * 128],
                                    idr_sb[:])
                        nc.scalar.activation(
                            ht[:, dt2 * 2 * CHUNK:(dt2 + 1) * 2 * CHUNK],
                            tp[:], AF.Copy)

                    # -- projections per (proj, head)
                    for head in range(HPC):
                        for kind, w_sb, dst in (("q", wq_sb, qt_d),
                                                ("k", wk_sb, kt_d),
                                                ("v", wv_sb, vt_d)):
                            pp = p1ps_p.tile([128, CHUNK], FP32, name="pp")
                            for dt in range(DT):
                                nc.tensor.matmul(
                                    pp[:],
                                    w_sb[:, dt * M + head * HD:
                                         dt * M + (head + 1) * HD],
                                    ht[:, dt * CHUNK:(dt + 1) * CHUNK],
                                    start=(dt == 0), stop=(dt == DT - 1))
                            if kind == "v":
                                vstage = p1st.tile([128, CHUNK], FP32R, name="vstage")
                                nc.scalar.activation(vstage[:], pp[:], AF.Copy)
                                nc.sync.dma_start(
                                    out=dst[head][:, c * CHUNK:(c + 1) * CHUNK],
                                    in_=vstage[:])
                            else:
                                # RoPE: pc = pp*cos (SBUF), ps = pp*sin (PSUM)
                                ps = p1ps_r.tile([128, CHUNK], FP32, name="ps")
                                nc.vector.tensor_tensor(
                                    out=ps[:], in0=pp[:],
                                    in1=sin_sb[:, c * CHUNK:(c + 1) * CHUNK],
                                    op=ALU.mult)
                                pc = p1st.tile([128, CHUNK], FP32, name="pc")
                                nc.vector.tensor_tensor(
                                    out=pc[:], in0=pp[:],
                                    in1=cos_sb[:, c * CHUNK:(c + 1) * CHUNK],
                                    op=ALU.mult)
                                qstage = p1st.tile([128, CHUNK], FP32R,
                                                   name="qstage")
                                nc.vector.tensor_tensor(
                                    out=qstage[0:64, :], in0=pc[0:64, :],
                                    in1=ps[64:128, :], op=ALU.subtract)
                                nc.vector.tensor_tensor(
                                    out=qstage[64:128, :], in0=pc[64:128, :],
                                    in1=ps[0:64, :], op=ALU.add)
                                nc.sync.dma_start(
                                    out=dst[head][:, c * CHUNK:(c + 1) * CHUNK],
                                    in_=qstage[:])

            # ---------------- Phase 2: attention per head (+ inlined output
            # projection during head 1, overlapping its PE/DMA with attention)
            with tc.tile_pool(name="p2a", bufs=1) as p2a, \
                 tc.tile_pool(name="p2c", bufs=3) as p2c:
                at0_sb = p2a.tile([128, SEQ], FP32R, name="at0_sb")
                wot = [p2a.tile([128, D], FP32R, name=f"wot{h}")
                       for h in range(HPC)]
                with tc.tile_pool(name="p2h", bufs=2) as p2h, \
                     tc.tile_pool(name="p2o", bufs=2) as p2o, \
                     tc.tile_pool(name="p2p", bufs=3) as p2p, \
                     tc.tile_pool(name="p2ps_s", bufs=4, space="PSUM") as p2ps_s, \
                     tc.tile_pool(name="p2ps_o", bufs=1, space="PSUM") as p2ps_o, \
                     tc.tile_pool(name="p2ps_z", bufs=1, space="PSUM") as p2ps_z, \
                     tc.tile_pool(name="p2ps_j", bufs=2, space="PSUM") as p2ps_j:
                    for head in range(HPC):
                        qt_sb = p2h.tile([128, SEQ], FP32R, name="qt_sb")
                        kt_sb = p2h.tile([128, SEQ], FP32R, name="kt_sb")
                        vt_sb = p2h.tile([128, SEQ], FP32R, name="vt_sb")
                        vn_sb = p2h.tile([128, SEQ], FP32R, name="vn_sb")
                        for cl in range(NCHUNK):
                            sl = slice(cl * CHUNK, (cl + 1) * CHUNK)
                            nc.sync.dma_start(out=qt_sb[:, sl], in_=qt_d[head][:, sl])
                            nc.sync.dma_start(out=kt_sb[:, sl], in_=kt_d[head][:, sl])
                            nc.sync.dma_start(out=vt_sb[:, sl], in_=vt_d[head][:, sl])
                            for jt4 in range(2):
                                jt0 = cl * 4 + jt4 * 2
                                vp = p2ps_s.tile([128, 256], FP32R, name="vp", tag="sp")
                                for half in range(2):
                                    jt = jt0 + half
                                    nc.tensor.transpose(
                                        vp[:, half * 128:(half + 1) * 128],
                                        vt_sb[:, jt * 128:(jt + 1) * 128],
                                        idr_sb[:])
                                nc.vector.tensor_copy(
                                    vn_sb[:, jt0 * 128:(jt0 + 2) * 128],
                                    vp[:])

                        if head == 0:
                            wo_sb = p2h.tile([128, DT * M], FP32R,
                                             name="wo_sb", tag="vt_sb")
                            nc.sync.dma_start(
                                out=wo_sb[:].rearrange("p (t m) -> p t m", t=DT),
                                in_=wo[:].rearrange("(t p) m -> p t m", p=128))
                            for mh in range(HPC):
                                for dt2 in range(DT // 2):
                                    wp = p2ps_s.tile([128, 256], FP32R,
                                                     name="wp", tag="sp")
                                    for half in range(2):
                                        dt = dt2 * 2 + half
                                        nc.tensor.transpose(
                                            wp[:, half * 128:(half + 1) * 128],
                                            wo_sb[:, dt * M + mh * HD:
                                                  dt * M + (mh + 1) * HD],
                                            idr_sb[:])
                                    nc.vector.tensor_copy(
                                        wot[mh][:, dt2 * 256:(dt2 + 1) * 256],
                                        wp[:])
                        for c in range(NCHUNK):
                            o_acc = p2ps_o.tile([128, CHUNK], FP32, name="o_acc")
                            z_acc = p2ps_z.tile([128, CHUNK], FP32, name="z_acc")
                            jmax = 4 * c + 3
                            # software-pipelined: Z/PV lag scores/exp by 2 so a
                            # late exp never stalls the in-order PE queue.
                            # Diagonal tiles go first: their tri-mask latency
                            # hides behind the full tiles that follow, and the
                            # r=0 tile writes all columns so start= covers the
                            # whole accumulator.
                            LAG = 2
                            j_order = list(range(4 * c, 4 * c + 4)) + \
                                list(range(0, 4 * c))
                            pts = {}
                            for jj in range(jmax + 1 + LAG):
                                if jj <= jmax:
                                    j = j_order[jj]
                                    r = j - 4 * c
                                    off = max(r, 0) * 128
                                    sp = p2ps_s.tile([128, CHUNK], FP32,
                                                     name="sp")
                                    nc.tensor.matmul(
                                        sp[:, off:],
                                        kt_sb[:, j * 128:(j + 1) * 128],
                                        qt_sb[:, c * CHUNK + off:(c + 1) * CHUNK],
                                        start=True, stop=True)
                                    pt = p2p.tile([128, CHUNK], FP32R,
                                                  name="pt", bufs=6)
                                    nc.scalar.activation(pt[:, off:],
                                                         sp[:, off:], AF.Exp,
                                                         scale=float(SM_SCALE))
                                    if r >= 0:
                                        nc.vector.tensor_tensor(
                                            out=pt[:, off:off + 128],
                                            in0=pt[:, off:off + 128],
                                            in1=tri_sb[:], op=ALU.mult)
                                    pts[j] = (pt, off)
                                if jj >= LAG:
                                    idx = jj - LAG
                                    j = j_order[idx]
                                    pt, off = pts.pop(j)
                                    nc.tensor.matmul(
                                        z_acc[:, off:], ones_sb[:],
                                        pt[:, off:],
                                        start=(idx == 0), stop=(idx == jmax),
                                        skip_group_check=True)
                                    nc.tensor.matmul(
                                        o_acc[:, off:],
                                        vn_sb[:, j * 128:(j + 1) * 128],
                                        pt[:, off:],
                                        start=(idx == 0), stop=(idx == jmax),
                                        skip_group_check=True)
                            rz = p2p.tile([128, CHUNK], FP32, name="rz")
                            nc.vector.reciprocal(rz[:], z_acc[:])
                            if head == 0:
                                nc.vector.tensor_tensor(
                                    out=at0_sb[:, c * CHUNK:(c + 1) * CHUNK],
                                    in0=o_acc[:], in1=rz[:], op=ALU.mult)
                            else:
                                at1 = p2c.tile([128, CHUNK], FP32R, name="at1")
                                nc.vector.tensor_tensor(
                                    out=at1[:], in0=o_acc[:], in1=rz[:],
                                    op=ALU.mult)
                                # inline output projection for s-tiles of chunk c
                                for st4 in range(4):
                                    st = c * 4 + st4
                                    ostage = p2o.tile([128, D], FP32,
                                                      name="ostage")
                                    for dc in range(4):
                                        op = p2ps_j.tile([128, CHUNK], FP32,
                                                         name="op")
                                        nc.tensor.matmul(
                                            op[:],
                                            at0_sb[:, st * 128:(st + 1) * 128],
                                            wot[0][:, dc * CHUNK:(dc + 1) * CHUNK],
                                            start=True, stop=False)
                                        nc.tensor.matmul(
                                            op[:],
                                            at1[:, st4 * 128:(st4 + 1) * 128],
                                            wot[1][:, dc * CHUNK:(dc + 1) * CHUNK],
                                            start=False, stop=True)
                                        if dc % 2 == 0:
                                            nc.scalar.activation(
                                                ostage[:, dc * CHUNK:(dc + 1) * CHUNK],
                                                op[:], AF.Copy)
                                        else:
                                            nc.vector.tensor_copy(
                                                ostage[:, dc * CHUNK:(dc + 1) * CHUNK],
                                                op[:])
                                    nc.sync.dma_start(
                                        out=out[st * 128:(st + 1) * 128, :],
                                        in_=ostage[:])
    nc.finalize()
    return nc


def _host_prep(xs, norm_w, wq, wk, wv, wo):
    """Fold norm_w into qkv weights, permute rope dims, build tables."""
    nw = norm_w.astype(np.float32)[:, None, None]
    perm = np.concatenate([np.arange(0, HD, 2), np.arange(1, HD, 2)])
    wq_p = (wq * nw)[:, :, perm]
    wk_p = (wk * nw)[:, :, perm]
    wv_n = wv * nw

    inv_freq = 1.0 / (ROPE_BASE ** (np.arange(0, HD, 2, dtype=np.float32) / HD))
    pos = np.arange(SEQ, dtype=np.float32)
    ang = pos[:, None] * inv_freq[None, :]          # [S, 64]
    cos_t = np.cos(ang).T.astype(np.float32)        # [64, S]
    sin_t = np.sin(ang).T.astype(np.float32)
    cosd = np.concatenate([cos_t, cos_t], 0)        # [128, S]
    sind = np.concatenate([sin_t, sin_t], 0)

    tri = np.triu(np.ones((128, 128), dtype=np.float32))  # t <= s valid
    ones = np.ones((128, 128), dtype=np.float32)
    ident = np.eye(128, dtype=np.float32)

    common = {
        "xs": np.ascontiguousarray(xs.astype(np.float32)),
        "cosd": np.ascontiguousarray(cosd),
        "sind": np.ascontiguousarray(sind),
        "tri": np.ascontiguousarray(tri),
        "ones": ones,
        "ident": ident,
        "identr": ident,
    }
    in_maps = []
    for core in range(NCORES):
        h0 = core * HPC
        sl = slice(h0, h0 + HPC)
        in_maps.append({
            **common,
            "wq": np.ascontiguousarray(
                wq_p[:, sl, :].reshape(D, M).astype(np.float32)),
            "wk": np.ascontiguousarray(
                wk_p[:, sl, :].reshape(D, M).astype(np.float32)),
            "wv": np.ascontiguousarray(
                wv_n[:, sl, :].reshape(D, M).astype(np.float32)),
            "wo": np.ascontiguousarray(
                wo[:, sl, :].reshape(D, M).astype(np.float32)),
        })
    return in_maps


def kernel(xs, norm_w, wq, wk, wv, wo):
    trace = bool(int(os.environ.get("KERNEL_TRACE", "0")))
    if trace:
        _inject_ntff_hook()
    from concourse.bass_utils import run_bass_kernel_spmd

    nc = _build_nc()
    in_maps = _host_prep(np.asarray(xs), np.asarray(norm_w), np.asarray(wq),
                         np.asarray(wk), np.asarray(wv), np.asarray(wo))
    res = run_bass_kernel_spmd(nc, in_maps, core_ids=list(range(NCORES)),
                               trace=trace)
    if trace and res.exec_time_ns is not None:
        print(f"HW exec time: {res.exec_time_ns} ns")
    acc = np.zeros((SEQ, D), dtype=np.float64)
    for r in res.results:
        acc += r["out"].astype(np.float64)
    return acc.astype(np.float32)


if __name__ == "__main__":
    rng = np.random.default_rng(0)
    scale = 1.0 / np.sqrt(D)
    inputs = {
        "xs": rng.standard_normal((SEQ, D), dtype=np.float32),
        "norm_w": np.ones((D,), np.float32),
        "wq": rng.standard_normal((D, NH, HD), dtype=np.float32) * scale,
        "wk": rng.standard_normal((D, NH, HD), dtype=np.float32) * scale,
        "wv": rng.standard_normal((D, NH, HD), dtype=np.float32) * scale,
        "wo": rng.standard_normal((D, NH, HD), dtype=np.float32) * scale,
    }
    out = kernel(**inputs)
    print(out.shape, out.dtype, float(np.abs(out).max()))
